# revision 2
# baseline (speedup 1.0000x reference)
"""Bass/Tile kernel for nn_MCA (multi-head cross-attention), 8-core SPMD.

Sharding: batch B(4) x head-group(2) -> 8 cores. Core c handles batch
b = c//2 and heads [g*8, (g+1)*8) where g = c%2. Each core computes a
partial output (T, C) = y_g @ Wu[:, g-cols].T; host sums the two
head-group partials per batch and adds bu.

Per-core pipeline (all matmuls bf16 -> fp32 PSUM):
  qT = (Wq_g @ x1b.T + bq)/sqrt(C)   (DG, T), d on partitions
  kT =  Wk_g @ x2b.T + bk            (DG, T)
  v  =  x2b @ Wv_g.T + bv            (T, DG), t on partitions, stored
                                      per-head with a ones column (65)
  per head, per q-block:
    S.T[k,q] = kT^h.T-slices @ qT^h  (k on partitions)  [K=64 matmuls,
               head pairs packed onto row-group halves of the PE array]
    P.T = exp(S.T)                   (ACT, evicts PSUM->SBUF bf16)
    O.T[0:64] = sum_k v_aug.T @ P.T  (ones column gives rowsum in row 64)
    yT = O.T[0:64] * (1/rowsum)      (DVE; recip row broadcast via DMA)
  out_partial = yT.T-slices @ WuT    (T, C) fp32
"""

import os
from contextlib import ExitStack

import numpy as np

BF16 = None  # set lazily in _imports
F32 = None

_PROGRAM_CACHE = {}


def _imports():
    import concourse.bass as bass
    import concourse.tile as tile
    from concourse import bacc, mybir
    from concourse.bass_utils import run_bass_kernel_spmd

    return bass, tile, bacc, mybir, run_bass_kernel_spmd


def build_program(T=2048, C=1024, HLOC=8, n_cores=8):
    """Build + compile the per-core Tile program (SPMD; same for all cores)."""
    bass, tile, bacc, mybir, _ = _imports()
    BF16 = mybir.dt.bfloat16
    F32 = mybir.dt.float32
    AF = mybir.ActivationFunctionType
    ALU = mybir.AluOpType

    hd = 64
    DG = HLOC * hd            # head-group feature dim (512)
    P = 128
    KT = C // P               # contraction tiles for projections (8)
    MT = DG // P              # d-tiles (4)
    NBLK = 512                # t-block width for projections / q-blocks
    NB = T // NBLK            # 4
    KTT = T // P              # key tiles in attention (16)
    HP = MT                   # head pairs == d-tiles
    scale = 1.0 / np.sqrt(C)

    nc = bacc.Bacc("TRN2", target_bir_lowering=False, debug=False,
                   num_devices=n_cores)

    xqT = nc.dram_tensor("xqT", [C, T], BF16, kind="ExternalInput").ap()
    xkT = nc.dram_tensor("xkT", [C, T], BF16, kind="ExternalInput").ap()
    wqT = nc.dram_tensor("wqT", [C, DG], BF16, kind="ExternalInput").ap()
    wkT = nc.dram_tensor("wkT", [C, DG], BF16, kind="ExternalInput").ap()
    wvT = nc.dram_tensor("wvT", [C, DG], BF16, kind="ExternalInput").ap()
    wuT = nc.dram_tensor("wuT", [DG, C], BF16, kind="ExternalInput").ap()
    bq = nc.dram_tensor("bq", [P, MT], F32, kind="ExternalInput").ap()
    bk = nc.dram_tensor("bk", [P, MT], F32, kind="ExternalInput").ap()
    bv = nc.dram_tensor("bv", [1, DG], F32, kind="ExternalInput").ap()
    out = nc.dram_tensor("out", [T, C], F32, kind="ExternalOutput").ap()

    with tile.TileContext(nc) as tc, ExitStack() as ctx:
        const = ctx.enter_context(tc.tile_pool(name="const", bufs=1))
        persist = ctx.enter_context(tc.tile_pool(name="persist", bufs=1))
        xpool = ctx.enter_context(tc.tile_pool(name="xpool", bufs=2))
        ps_proj = ctx.enter_context(tc.tile_pool(name="ps_proj", bufs=2,
                                                 space="PSUM"))
        ps_s = ctx.enter_context(tc.tile_pool(name="ps_s", bufs=2,
                                              space="PSUM"))
        ps_o = ctx.enter_context(tc.tile_pool(name="ps_o", bufs=2,
                                              space="PSUM"))
        ppool = ctx.enter_context(tc.tile_pool(name="ppool", bufs=4))
        epil = ctx.enter_context(tc.tile_pool(name="epil", bufs=4))
        outp = ctx.enter_context(tc.tile_pool(name="outp", bufs=3))

        # ---- constants / weights resident in SBUF ----
        wq_sb = persist.tile([P, KT, DG], BF16)
        wk_sb = persist.tile([P, KT, DG], BF16)
        wv_sb = persist.tile([P, KT, DG], BF16)
        wu_sb = persist.tile([P, MT, C], BF16)
        nc.sync.dma_start(out=wq_sb[:], in_=wqT.rearrange("(k p) d -> p k d", p=P))
        nc.sync.dma_start(out=wk_sb[:], in_=wkT.rearrange("(k p) d -> p k d", p=P))
        nc.sync.dma_start(out=wv_sb[:], in_=wvT.rearrange("(k p) d -> p k d", p=P))
        nc.sync.dma_start(out=wu_sb[:], in_=wuT.rearrange("(k p) d -> p k d", p=P))

        bq_sb = const.tile([P, MT], F32)
        bk_sb = const.tile([P, MT], F32)
        nc.sync.dma_start(out=bq_sb[:], in_=bq)
        nc.sync.dma_start(out=bk_sb[:], in_=bk)
        # bv broadcast to all 128 partitions once (used during v eviction)
        bv_bc = const.tile([P, DG], F32)
        nc.sync.dma_start(out=bv_bc[:], in_=bv.partition_broadcast(P))

        # persistent activations
        qT_sb = persist.tile([P, MT, T], BF16)
        kT_sb = persist.tile([P, MT, T], BF16)
        v_aug = persist.tile([P, KTT, HLOC * (hd + 1)], BF16)
        yT_sb = persist.tile([P, MT, T], BF16)

        # ones column for the rowsum trick: fill v_aug with 1.0 once;
        # evictions overwrite the 64-wide head slices, col 64 stays 1.0
        nc.gpsimd.memset(v_aug[:], 1.0)

        xq_r = xqT.rearrange("(k p) t -> p k t", p=P)
        xk_r = xkT.rearrange("(k p) t -> p k t", p=P)

        # ---- phase 1: projections ----
        for nt in range(NB):
            tsl = slice(nt * NBLK, (nt + 1) * NBLK)
            xq_t = xpool.tile([P, KT, NBLK], BF16)
            nc.sync.dma_start(out=xq_t[:], in_=xq_r[:, :, tsl])
            xk_t = xpool.tile([P, KT, NBLK], BF16)
            nc.sync.dma_start(out=xk_t[:], in_=xk_r[:, :, tsl])

            # qT and kT: out tiles (128 d, NBLK t)
            for m in range(MT):
                msl = slice(m * P, (m + 1) * P)
                ps = ps_proj.tile([P, NBLK], F32, tag="ps")
                for k in range(KT):
                    nc.tensor.matmul(ps[:], wq_sb[:, k, msl], xq_t[:, k, :],
                                     start=(k == 0), stop=(k == KT - 1))
                nc.vector.tensor_scalar(
                    out=qT_sb[:, m, tsl], in0=ps[:],
                    scalar1=bq_sb[:, m:m + 1], scalar2=scale,
                    op0=ALU.add, op1=ALU.mult)
            for m in range(MT):
                msl = slice(m * P, (m + 1) * P)
                ps = ps_proj.tile([P, NBLK], F32, tag="ps")
                for k in range(KT):
                    nc.tensor.matmul(ps[:], wk_sb[:, k, msl], xk_t[:, k, :],
                                     start=(k == 0), stop=(k == KT - 1))
                nc.vector.tensor_scalar(
                    out=kT_sb[:, m, tsl], in0=ps[:],
                    scalar1=bk_sb[:, m:m + 1], scalar2=None,
                    op0=ALU.add)

            # v: out tiles (128 t, DG d); t-subtiles of this block
            for m in range(NBLK // P):
                tidx = nt * (NBLK // P) + m
                msl = slice(m * P, (m + 1) * P)
                ps = ps_proj.tile([P, DG], F32, tag="ps")
                for k in range(KT):
                    nc.tensor.matmul(ps[:], xk_t[:, k, msl], wv_sb[:, k, :],
                                     start=(k == 0), stop=(k == KT - 1))
                nc.vector.tensor_add(
                    v_aug[:, tidx].rearrange("p (h e) -> p h e", e=hd + 1)[:, :, 0:hd],
                    ps[:].rearrange("p (h e) -> p h e", e=hd),
                    bv_bc[:].rearrange("p (h e) -> p h e", e=hd))

        # ---- phase 2: attention ----
        for hp in range(HP):
            hA, hB = 2 * hp, 2 * hp + 1
            for qb in range(NB):
                qsl = slice(qb * NBLK, (qb + 1) * NBLK)
                o_A = ps_o.tile([hd + 1, NBLK], F32, tag="o")
                o_B = ps_o.tile([hd + 1, NBLK], F32, tag="o")
                for ktp in range(KTT // 2):
                    s_A = ps_s.tile([P, 2, NBLK], F32, tag="s")
                    s_B = ps_s.tile([P, 2, NBLK], F32, tag="s")
                    for j in range(2):
                        kt = 2 * ktp + j
                        ksl = slice(kt * P, (kt + 1) * P)
                        # head pair packed on PE row-group halves
                        nc.tensor.matmul(s_A[:, j, :],
                                         kT_sb[0:hd, hp, ksl],
                                         qT_sb[0:hd, hp, qsl],
                                         start=True, stop=True)
                        nc.tensor.matmul(s_B[:, j, :],
                                         kT_sb[hd:P, hp, ksl],
                                         qT_sb[hd:P, hp, qsl],
                                         start=True, stop=True)
                    p_A = ppool.tile([P, 2, NBLK], BF16, tag="p")
                    nc.scalar.activation(p_A[:], s_A[:], AF.Exp)
                    p_B = ppool.tile([P, 2, NBLK], BF16, tag="p")
                    nc.scalar.activation(p_B[:], s_B[:], AF.Exp)
                    for j in range(2):
                        kt = 2 * ktp + j
                        nc.tensor.matmul(
                            o_A[:], v_aug[:, kt, hA * (hd + 1):(hA + 1) * (hd + 1)],
                            p_A[:, j, :], start=(kt == 0), stop=(kt == KTT - 1))
                        nc.tensor.matmul(
                            o_B[:], v_aug[:, kt, hB * (hd + 1):(hB + 1) * (hd + 1)],
                            p_B[:, j, :], start=(kt == 0), stop=(kt == KTT - 1))
                for o_t, prow in ((o_A, slice(0, hd)), (o_B, slice(hd, P))):
                    recip = epil.tile([1, NBLK], F32, tag="recip")
                    nc.vector.reciprocal(recip[:], o_t[hd:hd + 1, :])
                    bcast = epil.tile([hd, NBLK], F32, tag="bcast")
                    nc.gpsimd.partition_broadcast(bcast[:], recip[:])
                    nc.vector.tensor_mul(yT_sb[prow, hp, qsl],
                                         o_t[0:hd, :], bcast[:])

        # ---- phase 3: output projection ----
        for qt in range(T // P):
            qsl = slice(qt * P, (qt + 1) * P)
            for jt in range(C // NBLK):
                jsl = slice(jt * NBLK, (jt + 1) * NBLK)
                ps = ps_proj.tile([P, NBLK], F32, tag="ps")
                for dt in range(MT):
                    nc.tensor.matmul(ps[:], yT_sb[:, dt, qsl],
                                     wu_sb[:, dt, jsl],
                                     start=(dt == 0), stop=(dt == MT - 1))
                o_sb = outp.tile([P, NBLK], F32, tag="osb")
                nc.vector.tensor_copy(o_sb[:], ps[:])
                nc.sync.dma_start(out=out[qsl, jsl], in_=o_sb[:])

    nc.compile()
    return nc


def _get_program():
    key = "main"
    if key not in _PROGRAM_CACHE:
        _PROGRAM_CACHE[key] = build_program()
    return _PROGRAM_CACHE[key]


def make_in_maps(x1, x2, Wq, bq, Wk, bk, Wv, bv, Wu, bu, n_cores=8):
    import ml_dtypes
    bf16 = ml_dtypes.bfloat16
    T, B, C = x1.shape
    H = 16
    DG = C // 2  # head-group feature dim (8 heads x 64)
    x1 = np.asarray(x1, np.float32)
    x2 = np.asarray(x2, np.float32)
    in_maps = []
    for core in range(n_cores):
        b, g = core // 2, core % 2
        gs = slice(g * DG, (g + 1) * DG)
        in_maps.append({
            "xqT": np.ascontiguousarray(x1[:, b, :].T).astype(bf16),
            "xkT": np.ascontiguousarray(x2[:, b, :].T).astype(bf16),
            "wqT": np.ascontiguousarray(np.asarray(Wq)[gs, :].T).astype(bf16),
            "wkT": np.ascontiguousarray(np.asarray(Wk)[gs, :].T).astype(bf16),
            "wvT": np.ascontiguousarray(np.asarray(Wv)[gs, :].T).astype(bf16),
            "wuT": np.ascontiguousarray(np.asarray(Wu)[:, gs].T).astype(bf16),
            "bq": np.ascontiguousarray(
                np.asarray(bq, np.float32)[gs].reshape(-1, 128).T),
            "bk": np.ascontiguousarray(
                np.asarray(bk, np.float32)[gs].reshape(-1, 128).T),
            "bv": np.asarray(bv, np.float32)[gs].reshape(1, DG),
        })
    return in_maps


def kernel(x1, x2, Wq, bq, Wk, bk, Wv, bv, Wu, bu, _results_hook=None):
    _, _, _, _, run_bass_kernel_spmd = _imports()
    T, B, C = x1.shape
    nc = _get_program()
    in_maps = make_in_maps(x1, x2, Wq, bq, Wk, bk, Wv, bv, Wu, bu)
    br = run_bass_kernel_spmd(nc, in_maps, list(range(8)))
    if _results_hook is not None:
        _results_hook(br)
    outs = [np.asarray(r["out"], np.float32) for r in br.results]
    bu = np.asarray(bu, np.float32)
    full = np.stack([outs[2 * b] + outs[2 * b + 1] for b in range(B)], axis=0)
    full += bu.reshape(1, 1, -1)
    return full.astype(np.float32)


# revision 5
# speedup vs baseline: 1.1561x; 1.1561x over previous
"""Bass/Tile kernel for nn_MCA (multi-head cross-attention), 8-core SPMD.

Sharding: batch B(4) x head-group(2) -> 8 cores. Core c handles batch
b = c//2 and heads [g*8, (g+1)*8) where g = c%2. Each core computes a
partial output (T, C) = y_g @ Wu[:, g-cols].T; host sums the two
head-group partials per batch and adds bu.

Per-core pipeline (all matmuls bf16 -> fp32 PSUM):
  qT = (Wq_g @ x1b.T + bq)/sqrt(C)   (DG, T), d on partitions
  kT =  Wk_g @ x2b.T + bk            (DG, T)
  v  =  x2b @ Wv_g.T + bv            (T, DG), t on partitions, stored
                                      per-head with a ones column (65)
  per head, per q-block:
    S.T[k,q] = kT^h.T-slices @ qT^h  (k on partitions)  [K=64 matmuls,
               head pairs packed onto row-group halves of the PE array]
    P.T = exp(S.T)                   (ACT, evicts PSUM->SBUF bf16)
    O.T[0:64] = sum_k v_aug.T @ P.T  (ones column gives rowsum in row 64)
    yT = O.T[0:64] * (1/rowsum)      (DVE; recip row broadcast via DMA)
  out_partial = yT.T-slices @ WuT    (T, C) fp32
"""

import os
from contextlib import ExitStack

import numpy as np

BF16 = None  # set lazily in _imports
F32 = None

_PROGRAM_CACHE = {}


def _imports():
    import concourse.bass as bass
    import concourse.tile as tile
    from concourse import bacc, mybir
    from concourse.bass_utils import run_bass_kernel_spmd

    return bass, tile, bacc, mybir, run_bass_kernel_spmd


def build_program(T=2048, C=1024, HLOC=8, n_cores=8):
    """Build + compile the per-core Tile program (SPMD; same for all cores)."""
    bass, tile, bacc, mybir, _ = _imports()
    BF16 = mybir.dt.bfloat16
    F32 = mybir.dt.float32
    AF = mybir.ActivationFunctionType
    ALU = mybir.AluOpType

    hd = 64
    DG = HLOC * hd            # head-group feature dim (512)
    P = 128
    KT = C // P               # contraction tiles for projections (8)
    MT = DG // P              # d-tiles (4)
    NBLK = 512                # t-block width for projections / q-blocks
    NB = T // NBLK            # 4
    KTT = T // P              # key tiles in attention (16)
    HP = MT                   # head pairs == d-tiles
    scale = 1.0 / np.sqrt(C)

    nc = bacc.Bacc("TRN2", target_bir_lowering=False, debug=False,
                   num_devices=n_cores)

    xqT = nc.dram_tensor("xqT", [C, T], BF16, kind="ExternalInput").ap()
    xkT = nc.dram_tensor("xkT", [C, T], BF16, kind="ExternalInput").ap()
    wqT = nc.dram_tensor("wqT", [C, DG], BF16, kind="ExternalInput").ap()
    wkT = nc.dram_tensor("wkT", [C, DG], BF16, kind="ExternalInput").ap()
    wvT = nc.dram_tensor("wvT", [C, DG], BF16, kind="ExternalInput").ap()
    wuT = nc.dram_tensor("wuT", [DG, C], BF16, kind="ExternalInput").ap()
    bq = nc.dram_tensor("bq", [P, MT], F32, kind="ExternalInput").ap()
    bk = nc.dram_tensor("bk", [P, MT], F32, kind="ExternalInput").ap()
    bv = nc.dram_tensor("bv", [1, DG], F32, kind="ExternalInput").ap()
    out = nc.dram_tensor("out", [T, C], F32, kind="ExternalOutput").ap()

    with tile.TileContext(nc) as tc, ExitStack() as ctx:
        const = ctx.enter_context(tc.tile_pool(name="const", bufs=1))
        persist = ctx.enter_context(tc.tile_pool(name="persist", bufs=1))
        xpool = ctx.enter_context(tc.tile_pool(name="xpool", bufs=2))
        ps_s = ctx.enter_context(tc.tile_pool(name="ps_s", bufs=3,
                                              space="PSUM"))
        ps_proj = ps_s  # shared pool+tag: 3 slots x 2 banks + ps_o 2 = 8 banks
        ps_o = ctx.enter_context(tc.tile_pool(name="ps_o", bufs=2,
                                              space="PSUM"))
        ppool = ctx.enter_context(tc.tile_pool(name="ppool", bufs=4))
        epil = ctx.enter_context(tc.tile_pool(name="epil", bufs=4))
        outp = ctx.enter_context(tc.tile_pool(name="outp", bufs=3))

        # ---- constants / weights resident in SBUF ----
        wq_sb = persist.tile([P, KT, DG], BF16)
        wk_sb = persist.tile([P, KT, DG], BF16)
        wv_sb = persist.tile([P, KT, DG], BF16)
        wu_sb = persist.tile([P, MT, C], BF16)
        nc.sync.dma_start(out=wq_sb[:], in_=wqT.rearrange("(k p) d -> p k d", p=P))
        nc.sync.dma_start(out=wk_sb[:], in_=wkT.rearrange("(k p) d -> p k d", p=P))
        nc.sync.dma_start(out=wv_sb[:], in_=wvT.rearrange("(k p) d -> p k d", p=P))
        nc.sync.dma_start(out=wu_sb[:], in_=wuT.rearrange("(k p) d -> p k d", p=P))

        bq_sb = const.tile([P, MT], F32)
        bk_sb = const.tile([P, MT], F32)
        nc.sync.dma_start(out=bq_sb[:], in_=bq)
        nc.sync.dma_start(out=bk_sb[:], in_=bk)
        # bv broadcast to all 128 partitions once (used during v eviction)
        bv_bc = const.tile([P, DG], F32)
        nc.sync.dma_start(out=bv_bc[:], in_=bv.partition_broadcast(P))

        # persistent activations
        qT_sb = persist.tile([P, MT, T], BF16)
        kT_sb = persist.tile([P, MT, T], BF16)
        v_aug = persist.tile([P, KTT, HLOC * (hd + 1)], BF16)
        yT_sb = persist.tile([P, MT, T], BF16)

        # ones column for the rowsum trick: fill v_aug with 1.0 once;
        # evictions overwrite the 64-wide head slices, col 64 stays 1.0
        nc.gpsimd.memset(v_aug[:], 1.0)

        xq_r = xqT.rearrange("(k p) t -> p k t", p=P)
        xk_r = xkT.rearrange("(k p) t -> p k t", p=P)

        # ---- phase 1: projections ----
        for nt in range(NB):
            tsl = slice(nt * NBLK, (nt + 1) * NBLK)
            xq_t = xpool.tile([P, KT, NBLK], BF16)
            nc.sync.dma_start(out=xq_t[:], in_=xq_r[:, :, tsl])
            xk_t = xpool.tile([P, KT, NBLK], BF16)
            nc.sync.dma_start(out=xk_t[:], in_=xk_r[:, :, tsl])

            # qT and kT: out tiles (128 d, NBLK t)
            for m in range(MT):
                msl = slice(m * P, (m + 1) * P)
                ps = ps_proj.tile([P, NBLK], F32, tag="s")
                for k in range(KT):
                    nc.tensor.matmul(ps[:], wq_sb[:, k, msl], xq_t[:, k, :],
                                     start=(k == 0), stop=(k == KT - 1))
                nc.vector.tensor_scalar(
                    out=qT_sb[:, m, tsl], in0=ps[:],
                    scalar1=bq_sb[:, m:m + 1], scalar2=scale,
                    op0=ALU.add, op1=ALU.mult)
            for m in range(MT):
                msl = slice(m * P, (m + 1) * P)
                ps = ps_proj.tile([P, NBLK], F32, tag="s")
                for k in range(KT):
                    nc.tensor.matmul(ps[:], wk_sb[:, k, msl], xk_t[:, k, :],
                                     start=(k == 0), stop=(k == KT - 1))
                nc.vector.tensor_scalar(
                    out=kT_sb[:, m, tsl], in0=ps[:],
                    scalar1=bk_sb[:, m:m + 1], scalar2=None,
                    op0=ALU.add)

            # v: out tiles (128 t, DG d); t-subtiles of this block
            for m in range(NBLK // P):
                tidx = nt * (NBLK // P) + m
                msl = slice(m * P, (m + 1) * P)
                ps = ps_proj.tile([P, DG], F32, tag="s")
                for k in range(KT):
                    nc.tensor.matmul(ps[:], xk_t[:, k, msl], wv_sb[:, k, :],
                                     start=(k == 0), stop=(k == KT - 1))
                nc.vector.tensor_add(
                    v_aug[:, tidx].rearrange("p (h e) -> p h e", e=hd + 1)[:, :, 0:hd],
                    ps[:].rearrange("p (h e) -> p h e", e=hd),
                    bv_bc[:].rearrange("p (h e) -> p h e", e=hd))

        # ---- phase 2: attention (software-pipelined: S(k+1) issued on PE
        # ahead of V(k) so PE has independent work while ACT runs exp(k)) ----
        for hp in range(HP):
            hA, hB = 2 * hp, 2 * hp + 1
            for qb in range(NB):
                qsl = slice(qb * NBLK, (qb + 1) * NBLK)
                o_A = ps_o.tile([hd + 1, NBLK], F32, tag="o")
                o_B = ps_o.tile([hd + 1, NBLK], F32, tag="o")

                def emit_s(ktp):
                    s_A = ps_s.tile([P, 2, NBLK], F32, tag="s")
                    s_B = ps_s.tile([P, 2, NBLK], F32, tag="s")
                    for j in range(2):
                        kt = 2 * ktp + j
                        ksl = slice(kt * P, (kt + 1) * P)
                        # head pair packed on PE row-group halves
                        nc.tensor.matmul(s_A[:, j, :],
                                         kT_sb[0:hd, hp, ksl],
                                         qT_sb[0:hd, hp, qsl],
                                         start=True, stop=True)
                        nc.tensor.matmul(s_B[:, j, :],
                                         kT_sb[hd:P, hp, ksl],
                                         qT_sb[hd:P, hp, qsl],
                                         start=True, stop=True)
                    p_A = ppool.tile([P, 2, NBLK], BF16, tag="p")
                    nc.scalar.activation(p_A[:], s_A[:], AF.Exp)
                    p_B = ppool.tile([P, 2, NBLK], BF16, tag="p")
                    nc.scalar.activation(p_B[:], s_B[:], AF.Exp)
                    return p_A, p_B

                p_cur = emit_s(0)
                for ktp in range(KTT // 2):
                    p_next = emit_s(ktp + 1) if ktp + 1 < KTT // 2 else None
                    p_A, p_B = p_cur
                    for j in range(2):
                        kt = 2 * ktp + j
                        nc.tensor.matmul(
                            o_A[:], v_aug[:, kt, hA * (hd + 1):(hA + 1) * (hd + 1)],
                            p_A[:, j, :], start=(kt == 0), stop=(kt == KTT - 1))
                        nc.tensor.matmul(
                            o_B[:], v_aug[:, kt, hB * (hd + 1):(hB + 1) * (hd + 1)],
                            p_B[:, j, :], start=(kt == 0), stop=(kt == KTT - 1))
                    p_cur = p_next
                for o_t, prow in ((o_A, slice(0, hd)), (o_B, slice(hd, P))):
                    recip = epil.tile([1, NBLK], F32, tag="recip")
                    nc.vector.reciprocal(recip[:], o_t[hd:hd + 1, :])
                    bcast = epil.tile([hd, NBLK], F32, tag="bcast")
                    nc.gpsimd.partition_broadcast(bcast[:], recip[:])
                    nc.vector.tensor_mul(yT_sb[prow, hp, qsl],
                                         o_t[0:hd, :], bcast[:])

        # ---- phase 3: output projection ----
        for qt in range(T // P):
            qsl = slice(qt * P, (qt + 1) * P)
            for jt in range(C // NBLK):
                jsl = slice(jt * NBLK, (jt + 1) * NBLK)
                ps = ps_proj.tile([P, NBLK], F32, tag="s")
                for dt in range(MT):
                    nc.tensor.matmul(ps[:], yT_sb[:, dt, qsl],
                                     wu_sb[:, dt, jsl],
                                     start=(dt == 0), stop=(dt == MT - 1))
                o_sb = outp.tile([P, NBLK], F32, tag="osb")
                nc.vector.tensor_copy(o_sb[:], ps[:])
                nc.sync.dma_start(out=out[qsl, jsl], in_=o_sb[:])

    nc.compile()
    return nc


def _get_program():
    key = "main"
    if key not in _PROGRAM_CACHE:
        _PROGRAM_CACHE[key] = build_program()
    return _PROGRAM_CACHE[key]


def make_in_maps(x1, x2, Wq, bq, Wk, bk, Wv, bv, Wu, bu, n_cores=8):
    import ml_dtypes
    bf16 = ml_dtypes.bfloat16
    T, B, C = x1.shape
    H = 16
    DG = C // 2  # head-group feature dim (8 heads x 64)
    x1 = np.asarray(x1, np.float32)
    x2 = np.asarray(x2, np.float32)
    in_maps = []
    for core in range(n_cores):
        b, g = core // 2, core % 2
        gs = slice(g * DG, (g + 1) * DG)
        in_maps.append({
            "xqT": np.ascontiguousarray(x1[:, b, :].T).astype(bf16),
            "xkT": np.ascontiguousarray(x2[:, b, :].T).astype(bf16),
            "wqT": np.ascontiguousarray(np.asarray(Wq)[gs, :].T).astype(bf16),
            "wkT": np.ascontiguousarray(np.asarray(Wk)[gs, :].T).astype(bf16),
            "wvT": np.ascontiguousarray(np.asarray(Wv)[gs, :].T).astype(bf16),
            "wuT": np.ascontiguousarray(np.asarray(Wu)[:, gs].T).astype(bf16),
            "bq": np.ascontiguousarray(
                np.asarray(bq, np.float32)[gs].reshape(-1, 128).T),
            "bk": np.ascontiguousarray(
                np.asarray(bk, np.float32)[gs].reshape(-1, 128).T),
            "bv": np.asarray(bv, np.float32)[gs].reshape(1, DG),
        })
    return in_maps


def kernel(x1, x2, Wq, bq, Wk, bk, Wv, bv, Wu, bu, _results_hook=None):
    _, _, _, _, run_bass_kernel_spmd = _imports()
    T, B, C = x1.shape
    nc = _get_program()
    in_maps = make_in_maps(x1, x2, Wq, bq, Wk, bk, Wv, bv, Wu, bu)
    br = run_bass_kernel_spmd(nc, in_maps, list(range(8)))
    if _results_hook is not None:
        _results_hook(br)
    outs = [np.asarray(r["out"], np.float32) for r in br.results]
    bu = np.asarray(bu, np.float32)
    full = np.stack([outs[2 * b] + outs[2 * b + 1] for b in range(B)], axis=0)
    full += bu.reshape(1, 1, -1)
    return full.astype(np.float32)


# revision 7
# speedup vs baseline: 1.3603x; 1.1766x over previous
"""Bass/Tile kernel for nn_MCA (multi-head cross-attention), 8-core SPMD.

Sharding: batch B(4) x head-group(2) -> 8 cores. Core c handles batch
b = c//2 and heads [g*8, (g+1)*8) where g = c%2. Each core computes a
partial output (T, C) = y_g @ Wu[:, g-cols].T; host sums the two
head-group partials per batch and adds bu.

Per-core pipeline (all matmuls bf16 -> fp32 PSUM):
  qT = (Wq_g @ x1b.T + bq)/sqrt(C)   (DG, T), d on partitions
  kT =  Wk_g @ x2b.T + bk            (DG, T)
  v  =  x2b @ Wv_g.T + bv            (T, DG), t on partitions, stored
                                      per-head with a ones column (65)
  per head, per q-block:
    S.T[k,q] = kT^h.T-slices @ qT^h  (k on partitions)  [K=64 matmuls,
               head pairs packed onto row-group halves of the PE array]
    P.T = exp(S.T)                   (ACT, evicts PSUM->SBUF bf16)
    O.T[0:64] = sum_k v_aug.T @ P.T  (ones column gives rowsum in row 64)
    yT = O.T[0:64] * (1/rowsum)      (DVE; recip row broadcast via DMA)
  out_partial = yT.T-slices @ WuT    (T, C) fp32
"""

import os
from contextlib import ExitStack

import numpy as np

BF16 = None  # set lazily in _imports
F32 = None

_PROGRAM_CACHE = {}


def _imports():
    import concourse.bass as bass
    import concourse.tile as tile
    from concourse import bacc, mybir
    from concourse.bass_utils import run_bass_kernel_spmd

    return bass, tile, bacc, mybir, run_bass_kernel_spmd


def build_program(T=2048, C=1024, HLOC=8, n_cores=8):
    """Build + compile the per-core Tile program (SPMD; same for all cores)."""
    bass, tile, bacc, mybir, _ = _imports()
    BF16 = mybir.dt.bfloat16
    F32 = mybir.dt.float32
    AF = mybir.ActivationFunctionType
    ALU = mybir.AluOpType

    hd = 64
    DG = HLOC * hd            # head-group feature dim (512)
    P = 128
    KT = C // P               # contraction tiles for projections (8)
    MT = DG // P              # d-tiles (4)
    NBLK = 512                # t-block width for projections / q-blocks
    NB = T // NBLK            # 4
    KTT = T // P              # key tiles in attention (16)
    HP = MT                   # head pairs == d-tiles
    scale = 1.0 / np.sqrt(C)

    nc = bacc.Bacc("TRN2", target_bir_lowering=False, debug=False,
                   num_devices=n_cores)

    xqT = nc.dram_tensor("xqT", [C, T], BF16, kind="ExternalInput").ap()
    xkT = nc.dram_tensor("xkT", [C, T], BF16, kind="ExternalInput").ap()
    wqT = nc.dram_tensor("wqT", [C, DG], BF16, kind="ExternalInput").ap()
    wkT = nc.dram_tensor("wkT", [C, DG], BF16, kind="ExternalInput").ap()
    wvT = nc.dram_tensor("wvT", [C, DG], BF16, kind="ExternalInput").ap()
    wuT = nc.dram_tensor("wuT", [DG, C], BF16, kind="ExternalInput").ap()
    bq = nc.dram_tensor("bq", [P, MT], F32, kind="ExternalInput").ap()
    bk = nc.dram_tensor("bk", [P, MT], F32, kind="ExternalInput").ap()
    bv = nc.dram_tensor("bv", [1, DG], F32, kind="ExternalInput").ap()
    out = nc.dram_tensor("out", [T, C], F32, kind="ExternalOutput").ap()

    with tile.TileContext(nc) as tc, ExitStack() as ctx:
        const = ctx.enter_context(tc.tile_pool(name="const", bufs=1))
        persist = ctx.enter_context(tc.tile_pool(name="persist", bufs=1))
        xpool = ctx.enter_context(tc.tile_pool(name="xpool", bufs=2))
        ps_s = ctx.enter_context(tc.tile_pool(name="ps_s", bufs=3,
                                              space="PSUM"))
        ps_proj = ps_s  # shared pool+tag: 3 slots x 2 banks + ps_o 2 = 8 banks
        ps_o = ctx.enter_context(tc.tile_pool(name="ps_o", bufs=2,
                                              space="PSUM"))
        ppool = ctx.enter_context(tc.tile_pool(name="ppool", bufs=4))
        epil = ctx.enter_context(tc.tile_pool(name="epil", bufs=4))
        outp = ctx.enter_context(tc.tile_pool(name="outp", bufs=3))

        # ---- constants / weights resident in SBUF ----
        wq_sb = persist.tile([P, KT, DG], BF16)
        wk_sb = persist.tile([P, KT, DG], BF16)
        wv_sb = persist.tile([P, KT, DG], BF16)
        wu_sb = persist.tile([P, MT, C], BF16)
        nc.sync.dma_start(out=wq_sb[:], in_=wqT.rearrange("(k p) d -> p k d", p=P))
        nc.sync.dma_start(out=wk_sb[:], in_=wkT.rearrange("(k p) d -> p k d", p=P))
        nc.sync.dma_start(out=wv_sb[:], in_=wvT.rearrange("(k p) d -> p k d", p=P))
        nc.sync.dma_start(out=wu_sb[:], in_=wuT.rearrange("(k p) d -> p k d", p=P))

        bq_sb = const.tile([P, MT], F32)
        bk_sb = const.tile([P, MT], F32)
        nc.sync.dma_start(out=bq_sb[:], in_=bq)
        nc.sync.dma_start(out=bk_sb[:], in_=bk)
        # bv broadcast to all 128 partitions once (used during v eviction)
        bv_bc = const.tile([P, DG], F32)
        nc.sync.dma_start(out=bv_bc[:], in_=bv.partition_broadcast(P))

        # persistent activations
        qT_sb = persist.tile([P, MT, T], BF16)
        kT_sb = persist.tile([P, MT, T], BF16)
        v_aug = persist.tile([P, KTT, HLOC * (hd + 1)], BF16)
        yT_sb = persist.tile([P, MT, T], BF16)

        # ones column for the rowsum trick: fill v_aug with 1.0 once;
        # evictions overwrite the 64-wide head slices, col 64 stays 1.0
        nc.gpsimd.memset(v_aug[:], 1.0)

        xq_r = xqT.rearrange("(k p) t -> p k t", p=P)
        xk_r = xkT.rearrange("(k p) t -> p k t", p=P)

        # ---- phase 1: projections ----
        for nt in range(NB):
            tsl = slice(nt * NBLK, (nt + 1) * NBLK)
            xq_t = xpool.tile([P, KT, NBLK], BF16)
            nc.sync.dma_start(out=xq_t[:], in_=xq_r[:, :, tsl])
            xk_t = xpool.tile([P, KT, NBLK], BF16)
            nc.sync.dma_start(out=xk_t[:], in_=xk_r[:, :, tsl])

            # qT and kT: out tiles (128 d, NBLK t)
            for m in range(MT):
                msl = slice(m * P, (m + 1) * P)
                ps = ps_proj.tile([P, NBLK], F32, tag="s")
                for k in range(KT):
                    nc.tensor.matmul(ps[:], wq_sb[:, k, msl], xq_t[:, k, :],
                                     start=(k == 0), stop=(k == KT - 1))
                nc.vector.tensor_scalar(
                    out=qT_sb[:, m, tsl], in0=ps[:],
                    scalar1=bq_sb[:, m:m + 1], scalar2=scale,
                    op0=ALU.add, op1=ALU.mult)
            for m in range(MT):
                msl = slice(m * P, (m + 1) * P)
                ps = ps_proj.tile([P, NBLK], F32, tag="s")
                for k in range(KT):
                    nc.tensor.matmul(ps[:], wk_sb[:, k, msl], xk_t[:, k, :],
                                     start=(k == 0), stop=(k == KT - 1))
                nc.vector.tensor_scalar(
                    out=kT_sb[:, m, tsl], in0=ps[:],
                    scalar1=bk_sb[:, m:m + 1], scalar2=None,
                    op0=ALU.add)

            # v: out tiles (128 t, DG d); t-subtiles of this block
            for m in range(NBLK // P):
                tidx = nt * (NBLK // P) + m
                msl = slice(m * P, (m + 1) * P)
                ps = ps_proj.tile([P, DG], F32, tag="s")
                for k in range(KT):
                    nc.tensor.matmul(ps[:], xk_t[:, k, msl], wv_sb[:, k, :],
                                     start=(k == 0), stop=(k == KT - 1))
                nc.vector.tensor_add(
                    v_aug[:, tidx].rearrange("p (h e) -> p h e", e=hd + 1)[:, :, 0:hd],
                    ps[:].rearrange("p (h e) -> p h e", e=hd),
                    bv_bc[:].rearrange("p (h e) -> p h e", e=hd))

        # ---- phase 2: attention (software-pipelined: S(k+1) issued on PE
        # ahead of V(k) so PE has independent work while ACT runs exp(k)) ----
        for hp in range(HP):
            hA, hB = 2 * hp, 2 * hp + 1
            for qb in range(NB):
                qsl = slice(qb * NBLK, (qb + 1) * NBLK)
                o_A = ps_o.tile([hd + 1, NBLK], F32, tag="o")
                o_B = ps_o.tile([hd + 1, NBLK], F32, tag="o")

                def emit_s(ktp):
                    s_A = ps_s.tile([P, 2, NBLK], F32, tag="s")
                    s_B = ps_s.tile([P, 2, NBLK], F32, tag="s")
                    for j in range(2):
                        kt = 2 * ktp + j
                        ksl = slice(kt * P, (kt + 1) * P)
                        # head pair packed on PE row-group halves
                        nc.tensor.matmul(s_A[:, j, :],
                                         kT_sb[0:hd, hp, ksl],
                                         qT_sb[0:hd, hp, qsl],
                                         start=True, stop=True)
                        nc.tensor.matmul(s_B[:, j, :],
                                         kT_sb[hd:P, hp, ksl],
                                         qT_sb[hd:P, hp, qsl],
                                         start=True, stop=True)
                    p_A = ppool.tile([P, 2, NBLK], BF16, tag="p")
                    nc.scalar.activation(p_A[:], s_A[:], AF.Exp)
                    p_B = ppool.tile([P, 2, NBLK], BF16, tag="p")
                    nc.scalar.activation(p_B[:], s_B[:], AF.Exp)
                    return p_A, p_B

                p_cur = emit_s(0)
                for ktp in range(KTT // 2):
                    p_next = emit_s(ktp + 1) if ktp + 1 < KTT // 2 else None
                    p_A, p_B = p_cur
                    for j in range(2):
                        kt = 2 * ktp + j
                        nc.tensor.matmul(
                            o_A[:], v_aug[:, kt, hA * (hd + 1):(hA + 1) * (hd + 1)],
                            p_A[:, j, :], start=(kt == 0), stop=(kt == KTT - 1))
                        nc.tensor.matmul(
                            o_B[:], v_aug[:, kt, hB * (hd + 1):(hB + 1) * (hd + 1)],
                            p_B[:, j, :], start=(kt == 0), stop=(kt == KTT - 1))
                    p_cur = p_next
                for o_t, prow in ((o_A, slice(0, hd)), (o_B, slice(hd, P))):
                    # fast PSUM->SBUF copy frees the o slot for the next
                    # iteration's V-matmuls; slow recip runs off-path
                    o_sb = epil.tile([hd + 1, NBLK], F32, tag="o_sb")
                    nc.vector.tensor_copy(o_sb[:], o_t[:])
                    recip = epil.tile([1, NBLK], F32, tag="recip")
                    nc.vector.reciprocal(recip[:], o_sb[hd:hd + 1, :])
                    bcast = epil.tile([hd, NBLK], F32, tag="bcast")
                    nc.gpsimd.partition_broadcast(bcast[:], recip[:])
                    nc.vector.tensor_mul(yT_sb[prow, hp, qsl],
                                         o_sb[0:hd, :], bcast[:])

        # ---- phase 3: output projection ----
        for qt in range(T // P):
            qsl = slice(qt * P, (qt + 1) * P)
            for jt in range(C // NBLK):
                jsl = slice(jt * NBLK, (jt + 1) * NBLK)
                ps = ps_proj.tile([P, NBLK], F32, tag="s")
                for dt in range(MT):
                    nc.tensor.matmul(ps[:], yT_sb[:, dt, qsl],
                                     wu_sb[:, dt, jsl],
                                     start=(dt == 0), stop=(dt == MT - 1))
                o_sb = outp.tile([P, NBLK], F32, tag="osb")
                nc.vector.tensor_copy(o_sb[:], ps[:])
                nc.sync.dma_start(out=out[qsl, jsl], in_=o_sb[:])

    nc.compile()
    return nc


def _get_program():
    key = "main"
    if key not in _PROGRAM_CACHE:
        _PROGRAM_CACHE[key] = build_program()
    return _PROGRAM_CACHE[key]


def make_in_maps(x1, x2, Wq, bq, Wk, bk, Wv, bv, Wu, bu, n_cores=8):
    import ml_dtypes
    bf16 = ml_dtypes.bfloat16
    T, B, C = x1.shape
    H = 16
    DG = C // 2  # head-group feature dim (8 heads x 64)
    x1 = np.asarray(x1, np.float32)
    x2 = np.asarray(x2, np.float32)
    in_maps = []
    for core in range(n_cores):
        b, g = core // 2, core % 2
        gs = slice(g * DG, (g + 1) * DG)
        in_maps.append({
            "xqT": np.ascontiguousarray(x1[:, b, :].T).astype(bf16),
            "xkT": np.ascontiguousarray(x2[:, b, :].T).astype(bf16),
            "wqT": np.ascontiguousarray(np.asarray(Wq)[gs, :].T).astype(bf16),
            "wkT": np.ascontiguousarray(np.asarray(Wk)[gs, :].T).astype(bf16),
            "wvT": np.ascontiguousarray(np.asarray(Wv)[gs, :].T).astype(bf16),
            "wuT": np.ascontiguousarray(np.asarray(Wu)[:, gs].T).astype(bf16),
            "bq": np.ascontiguousarray(
                np.asarray(bq, np.float32)[gs].reshape(-1, 128).T),
            "bk": np.ascontiguousarray(
                np.asarray(bk, np.float32)[gs].reshape(-1, 128).T),
            "bv": np.asarray(bv, np.float32)[gs].reshape(1, DG),
        })
    return in_maps


def kernel(x1, x2, Wq, bq, Wk, bk, Wv, bv, Wu, bu, _results_hook=None):
    _, _, _, _, run_bass_kernel_spmd = _imports()
    T, B, C = x1.shape
    nc = _get_program()
    in_maps = make_in_maps(x1, x2, Wq, bq, Wk, bk, Wv, bv, Wu, bu)
    br = run_bass_kernel_spmd(nc, in_maps, list(range(8)))
    if _results_hook is not None:
        _results_hook(br)
    outs = [np.asarray(r["out"], np.float32) for r in br.results]
    bu = np.asarray(bu, np.float32)
    full = np.stack([outs[2 * b] + outs[2 * b + 1] for b in range(B)], axis=0)
    full += bu.reshape(1, 1, -1)
    return full.astype(np.float32)
